# revision 27
# baseline (speedup 1.0000x reference)
"""Causal self-attention on 8 Trainium2 NeuronCores (SPMD, no collectives).

Sharding (hardcoded): core c -> batch b = c//4, head-group g = c%4
(4 heads = 256 cols of Wq/Wk/Wv, 256 rows of Wo). Each core computes a
partial output  attended(heads of g) @ Wo[rows of g]  for its batch;
the host sums the 4 partials per batch (row-parallel unshard).

Device algorithm per core:
  - QT/KT/VT = W^T x^T projections ([d', s] layout; x^T pre-transposed on
    host). Q/K path stays float32r for score precision; the V/attention-
    output path (V tiles, exp'd weights, attended, Wo, partial out) is
    bf16 - same PE rate, half the DMA/SBUF, 2x/4x DVE modes.
  - per head: scores^T[k,q] = K Q^T (causal pieces only), exp on ScalarE
    (scale=1/8 folded into the activation), tri-mask on the diagonal block
  - AV with stationary [V | 1] so one accumulation yields attended^T AND
    the softmax denominator row; normalize via reciprocal + DMA partition
    broadcast (DRAM bounce), off the accumulators' critical path
  - partial out = stack(attn)^T @ Wo rows

Schedule: one 4-tag PSUM pool (A-D, [128,1024] f32 = 2 banks each) is
timeshared by projections (2 combos x 2 tiles), attention (sc_e, sc_o,
att_e, att_o) and the Wo blocks (pw). Emission order interleaves phases
to keep PE dense and spread the ACT exp load:
  P1=[(0,wq),(0,wk)]  P2=[(0,wv),(1,wq)]  vtiles(0)  attn(m0,qh0)
  P3=[(1,wk),(1,wv)]  vtiles(1)  attn(m0,qh1)  attn(m1,qh0)
  wo(0..7)  attn(m1,qh1)  wo(8..15)
Within attn(qh,m) the two parities (heads) are interleaved and the AV
matmuls lag the score matmuls by one k-block, so the PE never waits for
the ACT exp of the tile it just scored.
"""

import numpy as np
from contextlib import ExitStack

import concourse.bass as bass
import concourse.bacc as bacc
import concourse.tile as tile
from concourse import mybir
from concourse.bass_utils import run_bass_kernel_spmd

B, S, D, H, HD = 2, 2048, 1024, 16, 64
NCORES = 8
GROUPS = 4            # head-groups (cores per batch)
WCOLS = D // GROUPS   # 256 = 4 heads per core
PB = 128              # partition block
NKB = S // PB         # 16 key blocks
NDC = D // PB         # 8 contraction chunks
QH = S // 2           # 1024, query-half size
f32 = mybir.dt.float32
f32r = mybir.dt.float32r
bf16 = mybir.dt.bfloat16
EXP = mybir.ActivationFunctionType.Exp
COPY = mybir.ActivationFunctionType.Copy

LAST_RESULTS = None   # BassKernelResults of the last kernel() call

# scheduling knobs (resolved by sim search; see _transcript notes)
P3_EVAC = "act"
OT_EVAC = "act"



def _pieces(qlo, qhi, step=512):
    out = []
    a = qlo
    while a < qhi:
        b = min(a + step, qhi)
        out.append((a, b))
        a = b
    return out


def _pieces_grid(qlo, qhi, step=512):
    """Pieces that never cross a `step`-aligned boundary (PSUM banks)."""
    out = []
    a = qlo
    while a < qhi:
        b = min(qhi, (a // step + 1) * step)
        out.append((a, b))
        a = b
    return out


def build_nc(reps=1):
    """reps>1 repeats the whole computation in one program - used by the
    bench to amortize the (huge, ~30-75ms) axon dispatch overhead and
    expose the true per-execution HW time as the slope vs reps."""
    nc = bacc.Bacc("TRN2")
    xT = nc.declare_dram_parameter("xT", [D, S], f32, isOutput=False)
    wq = nc.declare_dram_parameter("wq", [D, WCOLS], f32, isOutput=False)
    wk = nc.declare_dram_parameter("wk", [D, WCOLS], f32, isOutput=False)
    wv = nc.declare_dram_parameter("wv", [D, WCOLS], f32, isOutput=False)
    wo = nc.declare_dram_parameter("wo", [WCOLS, D], bf16, isOutput=False)
    msk = nc.declare_dram_parameter("msk", [PB, PB], bf16, isOutput=False)
    idn = nc.declare_dram_parameter("idn", [PB, 64], bf16, isOutput=False)
    outp = nc.declare_dram_parameter("outp", [S, D], bf16, isOutput=True)

    with tile.TileContext(nc) as tc:
        for r in range(reps):
            with ExitStack() as ctx:
                _build(ctx, tc, nc, xT, wq, wk, wv, wo, msk, idn, outp,
                       sfx=f"_{r}" if reps > 1 else "")
    nc.compile()
    return nc


def _build(ctx, tc, nc, xT, wq, wk, wv, wo, msk, idn, outp, sfx=""):
    cst = ctx.enter_context(tc.tile_pool(name=f"cst{sfx}", bufs=1))
    qkv = ctx.enter_context(tc.tile_pool(name=f"qkv{sfx}", bufs=1))
    v1p = ctx.enter_context(tc.tile_pool(name=f"v1p{sfx}", bufs=1))
    dramp = ctx.enter_context(tc.tile_pool(name=f"dramp{sfx}", bufs=2,
                                           space="DRAM"))
    ldtmp = ctx.enter_context(tc.tile_pool(name=f"ldtmp{sfx}", bufs=3))
    expp = ctx.enter_context(tc.tile_pool(name=f"expp{sfx}", bufs=6))
    rp = ctx.enter_context(tc.tile_pool(name=f"rp{sfx}", bufs=2))
    op = ctx.enter_context(tc.tile_pool(name=f"op{sfx}", bufs=3))
    # the one PSUM pool: 4 double-bank slots timeshared by all phases
    psum = ctx.enter_context(tc.tile_pool(name=f"psum{sfx}", bufs=1,
                                          space="PSUM"))
    PT = ("A", "B", "C", "D")

    # --- f32r weights: DMA to staging, then a copy that rounds to
    # float32r (the PE requires rounded fp32r matmul operands).
    # chunked=True DMAs+rounds per 128-row chunk so the first projection
    # matmul only waits for chunk 0, not the whole weight ---
    def _load_w(name, h, shape, rearr, chunked=False):
        t = cst.tile(shape, f32r, tag=name, name=name)
        if chunked:
            for c in range(NDC):
                wt = ldtmp.tile([PB, WCOLS], f32, tag="ldw",
                                name=f"{name}_st{c}")
                nc.sync.dma_start(out=wt, in_=h[c * PB:(c + 1) * PB, :])
                nc.vector.tensor_copy(t[:, c, :], wt)
        else:
            wt = ldtmp.tile(shape, f32, tag="ldt", name=f"{name}_st")
            nc.sync.dma_start(out=wt, in_=h[:].rearrange(rearr, p=PB))
            nc.vector.tensor_copy(t, wt)
        return t

    # bf16 scratch columns for the [V | 1] stationaries
    onec = cst.tile([PB, 1], bf16, tag="onec", name="onec")
    nc.vector.memset(onec, 1.0)
    zoc = cst.tile([PB, 64], bf16, tag="zoc", name="zoc")
    nc.vector.memset(zoc, 0.0)
    nc.vector.memset(zoc[:, 32:33], 1.0)
    # [ones; zeros] rows for the PE partition-broadcast (fast normalize):
    # a 1-partition matmul fails the walrus ISA check (s3d3_mm_num_elements)
    # so use 2 partitions with a zero row to ignore the second input row
    ones_bc = cst.tile([66, PB], bf16, tag="ones_bc", name="ones_bc")
    nc.vector.memset(ones_bc, 0.0)
    nc.vector.memset(ones_bc[32:33, :], 1.0)
    nc.vector.memset(ones_bc[64:65, :], 1.0)

    qt, kt, vt = {}, {}, {}
    v1e, v1o = {}, {}
    stacks = {}

    xtp_ctx = ExitStack()
    xtp = xtp_ctx.enter_context(tc.tile_pool(name=f"xtp{sfx}", bufs=1))
    w_sb = {}
    xts = []

    def _load_xt(c):
        # halved DMA; rounding split ACT/DVE so neither engine gates the
        # chunk supply to the projection matmuls
        t = xtp.tile([PB, S], f32r, tag=f"xt{c}", name=f"xt{c}")
        for hi, (a, b) in enumerate(((0, S // 2), (S // 2, S))):
            st = ldtmp.tile([PB, S // 2], f32, tag="ldx", name=f"xt{c}_st")
            nc.sync.dma_start(out=st, in_=xT[c * PB:(c + 1) * PB, a:b])
            if hi == 0:
                nc.scalar.activation(t[:, a:b], st, COPY)
            else:
                nc.vector.tensor_copy(t[:, a:b], st)
        xts.append(t)

    # DMA order: xt0 + the weights needed first, then the rest of xT
    # interleaved, so the first matmuls start early.
    w_sb["wq"] = _load_w("wq", wq, [PB, NDC, WCOLS], "(c p) m -> p c m",
                         chunked=True)
    _load_xt(0)
    w_sb["wk"] = _load_w("wk", wk, [PB, NDC, WCOLS], "(c p) m -> p c m",
                         chunked=True)
    _load_xt(1)
    _load_xt(2)
    w_sb["wv"] = _load_w("wv", wv, [PB, NDC, WCOLS], "(c p) m -> p c m",
                         chunked=True)
    for c in range(3, NDC):
        _load_xt(c)
    # bf16 tensors come straight from DRAM - no rounding copy
    wo_sb = cst.tile([PB, 2, D], bf16, tag="wo", name="wo_sb")
    nc.sync.dma_start(out=wo_sb, in_=wo[:].rearrange("(r p) n -> p r n", p=PB))
    msk_sb = cst.tile([PB, PB], bf16, tag="msk", name="msk_sb")
    nc.sync.dma_start(out=msk_sb, in_=msk[:])
    idn_sb = cst.tile([PB, 64], bf16, tag="idn", name="idn_sb")
    nc.sync.dma_start(out=idn_sb, in_=idn[:])

    for m, wname in [(m, w) for m in range(2) for w in ("wq", "wk", "wv")]:
        store = {"wq": qt, "wk": kt, "wv": vt}[wname]
        dt = bf16 if wname == "wv" else f32r
        store[m] = qkv.tile([PB, S], dt, tag=f"{wname}{m}",
                            name=f"{wname}t{m}")

    def _proj_pair(combos, evac="act"):
        # c-outer accumulation: PE follows the xT DMA stream chunk by chunk.
        # 2 combos x 2 double-bank psum tiles = all 8 banks.
        pps = {}
        for j, (m, wname) in enumerate(combos):
            for pi in range(2):
                pps[(m, wname, pi)] = psum.tile(
                    [PB, 1024], f32, tag=PT[2 * j + pi], name="pp")
        for c in range(NDC):
            for m, wname in combos:
                for pi in range(2):
                    for (a, b) in _pieces(1024 * pi, 1024 * (pi + 1)):
                        nc.tensor.matmul(
                            pps[(m, wname, pi)][:, a - 1024 * pi:b - 1024 * pi],
                            w_sb[wname][:, c, m * PB:(m + 1) * PB],
                            xts[c][:, a:b],
                            start=(c == 0), stop=(c == NDC - 1),
                        )
        for m, wname in combos:
            dst = {"wq": qt, "wk": kt, "wv": vt}[wname][m]
            for pi in range(2):
                # evacuate on whichever engine has slack in this phase
                if evac == "act":
                    nc.scalar.activation(
                        dst[:, 1024 * pi:1024 * (pi + 1)],
                        pps[(m, wname, pi)], COPY)
                else:
                    nc.vector.tensor_copy(
                        dst[:, 1024 * pi:1024 * (pi + 1)],
                        pps[(m, wname, pi)])

    def _v_tiles(m):
        # V tiles [k-block, head-dim] bf16 with the ones column baked in
        for i in range(NKB):
            for parity in range(2):
                off = 64 * parity
                tp = psum.tile([PB, 64], bf16, tag=PT[parity], name="tp")
                nc.tensor.transpose(
                    tp,
                    vt[m][off:off + 64, i * PB:(i + 1) * PB],
                    idn_sb[off:off + 64, :],
                )
                if parity == 0:
                    ve = v1p.tile([PB, 65], bf16, tag=f"v1e{m}_{i}",
                                  name=f"v1e{m}_{i}")
                    nc.vector.tensor_copy(ve[:, 0:64], tp)
                    nc.vector.tensor_copy(ve[:, 64:65], onec)
                    v1e[(m, i)] = ve
                else:
                    vo = v1p.tile([PB, PB], bf16, tag=f"v1o{m}_{i}",
                                  name=f"v1o{m}_{i}")
                    nc.vector.tensor_copy(vo[:, 0:64], zoc)
                    nc.vector.tensor_copy(vo[:, 64:128], tp)
                    v1o[(m, i)] = vo

    def _attn(m, qh, fast_norm=False):
        """Both parities interleaved; AV lags scores by one k-block."""
        qbase = QH * qh
        nkb = (qh + 1) * (QH // PB)
        atts = {
            0: psum.tile([65, QH], f32, tag=PT[2], name="att_e"),
            1: psum.tile([PB, QH], f32, tag=PT[3], name="att_o"),
        }
        pend = None                   # (i, lo, {parity: ep})

        def _flush(last_i):
            i, lo, eps = pend
            for parity in range(2):
                v1t = v1e[(m, i)] if parity == 0 else v1o[(m, i)]
                for (a, b) in _pieces_grid(lo, qbase + QH, 512):
                    nc.tensor.matmul(
                        atts[parity][:, a - qbase:b - qbase],
                        v1t,
                        eps[parity][:, a - qbase:b - qbase],
                        start=(i == 0),
                        stop=(i == last_i),
                        skip_group_check=True,
                    )

        for i in range(nkb):
            qlo = max(PB * i, qbase)
            eps = {}
            for parity in range(2):
                off = 64 * parity
                sc = psum.tile([PB, QH], f32, tag=PT[parity], name="sc")
                for (a, b) in _pieces_grid(qlo, qbase + QH, 512):
                    nc.tensor.matmul(
                        sc[:, a - qbase:b - qbase],
                        kt[m][off:off + 64, i * PB:(i + 1) * PB],
                        qt[m][off:off + 64, a:b],
                        start=True, stop=True,
                    )
                ep = expp.tile([PB, QH], bf16, tag="ep", name="ep")
                nc.scalar.activation(
                    ep[:, qlo - qbase:], sc[:, qlo - qbase:], EXP, scale=0.125)
                if qlo == PB * i:     # starts at the diagonal block
                    nc.vector.tensor_mul(
                        ep[:, qlo - qbase:qlo - qbase + PB],
                        ep[:, qlo - qbase:qlo - qbase + PB],
                        msk_sb,
                    )
                eps[parity] = ep
            if pend is not None:
                _flush(nkb - 1)
            pend = (i, qlo, eps)
        _flush(nkb - 1)

        # normalize: attended rows / denominator row, via reciprocal +
        # DMA partition-broadcast (DRAM bounce; gpsimd partition_broadcast
        # mis-broadcasts from sliced partition bases on real hardware)
        if qh == 0:
            # reuse the (now dead) vt[m] slot; allocated per-m so vt[1] is
            # not re-slotted before _v_tiles(1) reads it
            stacks[m] = qkv.tile([PB, S], bf16, tag=f"wv{m}",
                                 name=f"stk{m}")
        for parity in range(2):
            drow = 64 if parity == 0 else 32
            rows = slice(0, 64) if parity == 0 else slice(64, 128)
            av = rp.tile([PB, QH], bf16, tag="av", name="av")
            if parity == 0:
                nc.vector.tensor_copy(av[0:65, :], atts[0][0:65, :])
            else:
                nc.vector.tensor_copy(av[64:128, :], atts[1][64:128, :])
                nc.vector.tensor_copy(av[32:33, :], atts[1][32:33, :])
            rt = rp.tile([66, QH], bf16, tag="rt", name="rt")
            if fast_norm:
                # zero the row below the denominator: it rides along in the
                # 2-partition broadcast matmul and must not contribute NaNs.
                # (memset both rows - engine partition bases must be 32-
                # aligned - then the reciprocal overwrites row drow)
                nc.vector.memset(rt[drow:drow + 2, :], 0.0)
            with nc.allow_low_precision(reason="1/denom at bf16: 0.4% rel, "
                                        "well inside the 2e-2 budget"):
                nc.vector.reciprocal(rt[drow:drow + 1, :],
                                     av[drow:drow + 1, :])
            if fast_norm:
                # tail of the kernel: broadcast 1/denom across partitions
                # with a 1-row ones matmul (low latency, PSUM is free now)
                rbp = psum.tile([PB, QH], f32, tag=PT[parity], name="rbp")
                for (a, b) in _pieces(0, QH):
                    nc.tensor.matmul(rbp[:, a:b], ones_bc[drow:drow + 2, :],
                                     rt[drow:drow + 2, a:b],
                                     start=True, stop=True)
                nc.vector.tensor_mul(
                    stacks[m][rows, qbase:qbase + QH], av[rows, :],
                    rbp[rows, :]
                )
                continue
            # off the critical path: partition-broadcast via a DRAM bounce
            # (HW-verified; the gpsimd partition_broadcast ucode
            # mis-broadcasts from sliced partition bases on real hardware)
            dr = dramp.tile([1, QH], bf16, tag="dr", name="dr")
            nc.sync.dma_start(out=dr, in_=rt[drow:drow + 1, :])
            rb = rp.tile([PB, QH], bf16, tag="rb", name="rb")
            bsrc = bass.AP(
                tensor=dr.tensor, offset=dr.offset,
                ap=[[0, 64]] + [list(d) for d in dr.ap[1:]],
            )
            nc.sync.dma_start(out=rb[rows, :], in_=bsrc)
            nc.vector.tensor_mul(
                stacks[m][rows, qbase:qbase + QH], av[rows, :], rb[rows, :]
            )

    def _wo_blocks(sbs):
        # partial = stack^T @ Wo_rows, for the given s-blocks
        for sb in sbs:
            pw = psum.tile([PB, D], f32, tag=PT[sb % 2], name="pw")
            for (a, b) in _pieces(0, D):
                for m in range(2):
                    nc.tensor.matmul(
                        pw[:, a:b],
                        stacks[m][:, sb * PB:(sb + 1) * PB],
                        wo_sb[:, m, a:b],
                        start=(m == 0), stop=(m == 1),
                    )
            ot = op.tile([PB, D], bf16, tag="ot", name="ot")
            if OT_EVAC == "dve" or (OT_EVAC == "alt" and sb % 2 == 1):
                nc.vector.tensor_copy(ot, pw)
            else:
                nc.scalar.activation(ot, pw, COPY)
            nc.sync.dma_start(out=outp[sb * PB:(sb + 1) * PB, :], in_=ot)

    # --- emission schedule: each _wo_blocks consumer is separated from
    # the normalize chains it reads by a full attention phase, except the
    # final one which uses the low-latency fast_norm path ---
    _proj_pair([(0, "wq"), (0, "wk")])
    _proj_pair([(0, "wv"), (1, "wq")])
    _v_tiles(0)
    _attn(0, 0)
    _proj_pair([(1, "wk"), (1, "wv")], evac=P3_EVAC)
    xtp_ctx.close()
    _v_tiles(1)
    _attn(1, 0)
    _attn(0, 1)
    _wo_blocks(range(8))
    _attn(1, 1, fast_norm=True)
    _wo_blocks(range(8, NKB))


def make_in_maps(x, Wq, Wk, Wv, Wo):
    import ml_dtypes
    x = np.ascontiguousarray(np.asarray(x, dtype=np.float32))
    Wq = np.ascontiguousarray(np.asarray(Wq, dtype=np.float32))
    Wk = np.ascontiguousarray(np.asarray(Wk, dtype=np.float32))
    Wv = np.ascontiguousarray(np.asarray(Wv, dtype=np.float32))
    Wo = np.ascontiguousarray(np.asarray(Wo, dtype=ml_dtypes.bfloat16))
    msk = np.triu(np.ones((PB, PB), dtype=ml_dtypes.bfloat16))
    idn = np.concatenate([np.eye(64)] * 2, axis=0).astype(ml_dtypes.bfloat16)
    in_maps = []
    for c in range(NCORES):
        b, g = divmod(c, GROUPS)
        in_maps.append({
            "xT": np.ascontiguousarray(x[b].T),
            "wq": np.ascontiguousarray(Wq[:, g * WCOLS:(g + 1) * WCOLS]),
            "wk": np.ascontiguousarray(Wk[:, g * WCOLS:(g + 1) * WCOLS]),
            "wv": np.ascontiguousarray(Wv[:, g * WCOLS:(g + 1) * WCOLS]),
            "wo": np.ascontiguousarray(Wo[g * WCOLS:(g + 1) * WCOLS, :]),
            "msk": msk,
            "idn": idn,
        })
    return in_maps


def _combine(outs):
    outs = [np.asarray(o).astype(np.float32) for o in outs]
    out = np.empty((B, S, D), dtype=np.float32)
    out[0] = outs[0] + outs[1] + outs[2] + outs[3]
    out[1] = outs[4] + outs[5] + outs[6] + outs[7]
    return out


def kernel(x, Wq, Wk, Wv, Wo):
    global LAST_RESULTS
    nc = build_nc()
    in_maps = make_in_maps(x, Wq, Wk, Wv, Wo)
    res = run_bass_kernel_spmd(nc, in_maps, list(range(NCORES)))
    LAST_RESULTS = res
    return _combine([r["outp"] for r in res.results])


def _make_runner(nc, in_maps):
    """Set up a device-resident one-dispatch runner for a prebuilt nc.

    Returns (run, fetch): run() executes one dispatch and returns wall
    seconds; fetch() returns the combined full-shape output of the last
    run."""
    import time
    import jax
    from jax.sharding import Mesh, NamedSharding, PartitionSpec
    from jax.experimental.shard_map import shard_map
    from concourse import bass2jax

    bass2jax.install_neuronx_cc_hook()

    partition_name = (
        nc.partition_id_tensor.name if nc.partition_id_tensor else None
    )
    in_names, out_names, out_avals, zero_outs = [], [], [], []
    for alloc in nc.m.functions[0].allocations:
        if not isinstance(alloc, mybir.MemoryLocationSet):
            continue
        name = alloc.memorylocations[0].name
        if alloc.kind == "ExternalInput":
            if name != partition_name:
                in_names.append(name)
        elif alloc.kind == "ExternalOutput":
            out_names.append(name)
            shape = tuple(alloc.tensor_shape)
            dtype = mybir.dt.np(alloc.dtype)
            out_avals.append(jax.core.ShapedArray(shape, dtype))
            zero_outs.append(np.zeros(shape, dtype))
    n_params = len(in_names)
    n_outs = len(out_names)
    all_names = in_names + out_names
    if partition_name is not None:
        all_names = all_names + [partition_name]

    def _body(*args):
        operands = list(args)
        if partition_name is not None:
            operands.append(bass2jax.partition_id_tensor())
        return tuple(bass2jax._bass_exec_p.bind(
            *operands,
            out_avals=tuple(out_avals),
            in_names=tuple(all_names),
            out_names=tuple(out_names),
            lowering_input_output_aliases=(),
            sim_require_finite=True,
            sim_require_nnan=True,
            nc=nc,
        ))

    devices = jax.devices()[:NCORES]
    mesh = Mesh(np.asarray(devices), ("core",))
    sharded = jax.jit(
        shard_map(_body, mesh=mesh,
                  in_specs=(PartitionSpec("core"),) * (n_params + n_outs),
                  out_specs=(PartitionSpec("core"),) * n_outs,
                  check_rep=False),
        donate_argnums=tuple(range(n_params, n_params + n_outs)),
        keep_unused=True,
    )
    sh = NamedSharding(mesh, PartitionSpec("core"))
    dev_in = [
        jax.device_put(
            np.concatenate(
                [np.asarray(in_maps[c][nm]) for c in range(NCORES)], axis=0),
            sh)
        for nm in in_names
    ]
    state = {"outs": None}

    def run():
        dev_zeros = [
            jax.device_put(
                np.zeros((NCORES * z.shape[0], *z.shape[1:]), z.dtype), sh)
            for z in zero_outs
        ]
        jax.block_until_ready(dev_zeros)
        jax.block_until_ready(dev_in)
        t0 = time.perf_counter()
        outs = sharded(*dev_in, *dev_zeros)
        jax.block_until_ready(outs)
        state["outs"] = outs
        return time.perf_counter() - t0

    def fetch():
        i = out_names.index("outp")
        arr = np.asarray(state["outs"][i]).reshape(NCORES, S, D)
        return _combine([arr[c] for c in range(NCORES)])

    return run, fetch


def bench(x, Wq, Wk, Wv, Wo, iters=8, nc=None):
    """Run the kernel with device-resident inputs; returns (out, times_s)."""
    if nc is None:
        nc = build_nc()
    run, fetch = _make_runner(nc, make_in_maps(x, Wq, Wk, Wv, Wo))
    times = [run() for _ in range(iters)]
    return fetch(), times


def bench_hw_time(x, Wq, Wk, Wv, Wo, reps=65, iters=24):
    """Measure the true per-execution HW time of the kernel.

    A single dispatch through the axon-proxied PJRT path costs ~30-110 ms
    of round-trip overhead regardless of the program (a trivial 1-tile
    copy kernel measures the same wall time as the full attention kernel),
    so single-call wall clock says nothing about device time. Instead, run
    one program containing the whole computation repeated `reps` times
    back-to-back on-device, and report the slope:

        hw_time = (min_wall(reps) - min_wall(1)) / (reps - 1)

    The reps=1 and reps=R dispatches are interleaved in one loop so both
    mins sample the same (drifting, long-tailed) RTT distribution; min
    over `iters` of each filters the noise.
    Returns (hw_time_s, out1, diag) where out1 is the reps=1 output.
    """
    nc1 = build_nc(reps=1)
    ncR = build_nc(reps=reps)
    in_maps = make_in_maps(x, Wq, Wk, Wv, Wo)
    run1, fetch1 = _make_runner(nc1, in_maps)
    runR, fetchR = _make_runner(ncR, in_maps)
    t1, tR = [], []
    for _ in range(iters):
        t1.append(run1())
        tR.append(runR())
    out1, outR = fetch1(), fetchR()
    m1, mR = min(t1), min(tR)
    hw_time = (mR - m1) / (reps - 1)
    diag = {"t1": t1, "tR": tR, "min1": m1, "minR": mR, "reps": reps,
            "outR": outR}
    return hw_time, out1, diag
